# revision 34
# baseline (speedup 1.0000x reference)
"""Trainium2 Bass kernel for nn_EnhanceSelfAttention (B=16, N=577, C=768, H=12).

Self-contained: takes full unsharded inputs, shards batch across 8 NeuronCores
(2 batches/core), runs a fused attention kernel per core, gathers the output.

Per-core pipeline (f16 matmul operands, fp32 PSUM accumulation):
  Host prep: xT (pre-transposed f16 x shard), qkv_w f16 with the q-columns
  pre-scaled by d^-0.5, q-bias pre-scaled likewise, out_b folded with the
  v-bias contribution (attn rows sum to 1, so v-bias commutes to out_b via
  qkv_b[2C:] @ out_w), expanded bias table t3m (Toeplitz gather, causal mask
  folded as -65504).

  C. v = xT.T @ wv per causal k-tile, [k, 12*65] f16 with a ones column
     per head (softmax-denominator trick).  psum->sbuf casts on DVE.
  G. gather bias: 24 strided DMAs from t3m (1.1KB lines) -> exp+reorder on
     ACT into per-k-tile [k, h-major] f16 exp(bias) tiles (masked -> 0).
  B. qT,kT = wqk.T @ xT per head-pair, psum->sbuf + bias on DVE
     (tensor_scalar_add, scale pre-folded into weights).
  D. per head-pair: sT = kT.T@qT, both heads into one 2-bank psum tile;
     one fused exp (ACT) -> es[k, 2, q]; one fused mult with the 2-head
     expb slice (DVE, 2x mode); OT += v.T@p over causal k-tiles in PSUM.
     Row 64 = denominator; partition_broadcast + divide on GpSimd.
  E. yT = out_w.T @ oT (dim-major output), psum->sbuf + out_b_eff bias on
     ACT, f16 yT streamed to DRAM; host transposes back.
"""

import numpy as np
import ml_dtypes

import concourse.bass as bass
import concourse.tile as tile
from concourse import bacc, mybir
from concourse.bass_utils import run_bass_kernel_spmd

F32 = mybir.dt.float32
BF16 = mybir.dt.bfloat16
F16 = mybir.dt.float16

B, NTOK, CDIM, NH, DH = 16, 577, 768, 12, 64
GRID = 24
NRD = (2 * GRID - 1) * (2 * GRID - 1) + 3  # 2212
NCORES = 8
BLOC = B // NCORES       # batches per core
NSEQ = BLOC * NTOK       # 1154
SCALE = DH ** -0.5       # 0.125
NEG = -65504.0

QBLOCKS = [(0, 121), (121, 456)]            # (qstart, qN)
# k-tiles: (k0, pw).  t=0: partitions 0..120 <-> k=0..120 (incl cls col k=0)
KTILES = [(0, 121), (121, 120), (241, 120), (361, 120), (481, 96)]
# per-tile stored q range: [QLO[t] .. 577)
QLO = [0, 121, 241, 361, 481]
WID = [NTOK - q for q in QLO]               # 577, 456, 336, 216, 96

# expanded-table strides (T3m[kh, d1, qh, h], d1 = qw-kw+23)
T3_KH = 47 * GRID * NH   # 13536
T3_D1 = GRID * NH        # 288

_CACHE = {}


def _check_rel_index(ri):
    """Assert the Toeplitz structure the gather DMAs rely on."""
    assert ri.shape == (NTOK, NTOK)
    assert ri[0, 0] == NRD - 1
    assert (ri[0, 1:] == NRD - 3).all()
    assert (ri[1:, 0] == NRD - 2).all()
    a = np.arange(NTOK - 1)
    qh, qw = a % GRID, a // GRID
    rel0 = qh[:, None] - qh[None, :] + GRID - 1
    rel1 = qw[:, None] - qw[None, :] + GRID - 1
    expect = rel0 + rel1 * (2 * GRID - 1)
    assert np.array_equal(ri[1:, 1:], expect), "rel_index lacks expected structure"


def _host_prep(pos_emb, rel_index):
    _check_rel_index(np.asarray(rel_index))
    pe_t = np.asarray(pos_emb, dtype=np.float32).T      # [NRD, NH]
    # expanded gather table with the causal mask folded in:
    # T3m[kh, d1, qh, h] = pos_emb[h, (qh-kh+23) + 47*d1] + (NEG if q<k)
    # where q-k = (qh-kh) + 24*(d1-23)
    kh = np.arange(GRID)[:, None, None]
    d1 = np.arange(2 * GRID - 1)[None, :, None]
    qh = np.arange(GRID)[None, None, :]
    ridx = (qh - kh + GRID - 1) + (2 * GRID - 1) * d1   # [24, 47, 24]
    t3m = pe_t[ridx]                                    # [24, 47, 24, NH]
    masked = (qh - kh) + GRID * (d1 - (GRID - 1)) < 0
    t3m = t3m + np.where(masked, NEG, 0.0)[..., None].astype(np.float32)
    t3m = np.ascontiguousarray(t3m.reshape(-1)).astype(ml_dtypes.bfloat16)
    pos_embT = np.ascontiguousarray(pe_t).reshape(-1)
    return t3m, pos_embT


def _build(ri):
    """Build + compile the per-core Bass program."""
    nc = bacc.Bacc("TRN2", target_bir_lowering=False, debug=False)

    xT_d = nc.dram_tensor("xT_in", [CDIM, NSEQ], F16, kind="ExternalInput").ap()
    qkvwh_d = nc.dram_tensor("qkv_w_h", [CDIM, 3 * CDIM], F16,
                             kind="ExternalInput").ap()
    qkb_d = nc.dram_tensor("qkb_prep", [2 * CDIM], F32,
                           kind="ExternalInput").ap()
    t3m_d = nc.dram_tensor("t3m", [GRID * 47 * GRID * NH], BF16,
                           kind="ExternalInput").ap()
    pe_d = nc.dram_tensor("pos_embT", [NRD * NH], F32, kind="ExternalInput").ap()
    outwh_d = nc.dram_tensor("out_w_h", [CDIM, CDIM], F16,
                             kind="ExternalInput").ap()
    outb_d = nc.dram_tensor("out_b_eff", [CDIM], F32, kind="ExternalInput").ap()
    y_d = nc.dram_tensor("yT", [CDIM, NSEQ], F16, kind="ExternalOutput").ap()

    with tile.TileContext(nc) as tc:
        _emit(nc, tc, xT_d, qkvwh_d, qkb_d, t3m_d, pe_d, outwh_d, outb_d, y_d)
    nc.compile()
    return nc


def _emit(nc, tc, xT_d, qkvw_d, qkb_d, t3m_d, pe_d, outw_d, outb_d, y_d):
    from contextlib import ExitStack

    NBLK = [(0, 386), (386, 384), (770, 384)]   # n-blocks (E out proj)
    # B blocks aligned to the batch boundary (577) so each reads one xT tile
    NBLK_B = [(0, 289), (289, 288), (577, 289), (866, 288)]

    with ExitStack() as top:
        persist = top.enter_context(tc.tile_pool(name="persist", bufs=1))
        consts = top.enter_context(tc.tile_pool(name="consts", bufs=1))
        wo_pool = top.enter_context(tc.tile_pool(name="wo", bufs=1))

        # ---- constants (tiny DMAs, SP queue, issued after wqk) ----
        # qk bias columns packed [128, 12] (host pre-scaled the q half)
        qkb = consts.tile([128, 12], F32, tag="qkb", name="qkb")
        # out_b_eff columns packed [128, 6]
        outb = consts.tile([128, 6], F32, tag="outb", name="outb")

        # ---- persistent activation storage (all f16) ----
        qT = [persist.tile([128, NSEQ], F16, tag=f"qT{j}", name=f"qT{j}")
              for j in range(6)]
        kT = [persist.tile([128, NSEQ], F16, tag=f"kT{j}", name=f"kT{j}")
              for j in range(6)]
        # per head: 64 value cols + 64 ones cols -> AV rows 64:128 hold the
        # softmax denominator already replicated across partitions (no
        # partition_broadcast needed before the divide)
        vt = [[persist.tile([128, NH * 128], F16, tag=f"v{b}_{t}",
                            name=f"v{b}_{t}")
               for t in range(5)] for b in range(BLOC)]
        oT = [persist.tile([128, NSEQ], F16, tag=f"oT{j}", name=f"oT{j}")
              for j in range(6)]
        expb = [persist.tile([128, WID[t] * NH], F16, tag=f"expb{t}",
                             name=f"expb{t}") for t in range(5)]
        wota = wo_pool.tile([128, 6 * CDIM], F16, tag="wota", name="wota")

        def wot(c):
            return wota[:, c * CDIM:(c + 1) * CDIM]

        # ================= phase 1: projections + gather =================
        with tc.tile_pool(name="xT", bufs=1) as xTp, \
             tc.tile_pool(name="wqk_pool", bufs=1) as wqk_pool, \
             tc.tile_pool(name="stage", bufs=1) as stagep:
            # all 6 row-chunks side by side per tile; xT split per batch and
            # wv per column-half so phase C's first groups depend on the
            # smallest possible DMA set
            xTb = [xTp.tile([128, 6 * NTOK], F16, tag=f"xT{b}", name=f"xT{b}")
                   for b in range(BLOC)]
            wqka = wqk_pool.tile([128, 6 * 1536], F16, tag="wqka", name="wqka")

            def xT(c, b):
                return xTb[b][:, c * NTOK:(c + 1) * NTOK]

            def wqk(c):
                return wqka[:, c * 1536:(c + 1) * 1536]

            # DMA order on SP queue: wv -> xT(b0) -> xT(b1) -> gathers ->
            # wqk -> consts -> wot.  xT split per batch so C(b0) starts early.
            with tc.tile_pool(name="wv_pool", bufs=1) as wv_pool, \
                 tc.tile_pool(name="warm", bufs=1) as warmp, \
                 tc.tile_pool(name="ps_w", bufs=1, space="PSUM") as ps_w, \
                 tc.tile_pool(name="ps_v", bufs=5, space="PSUM") as ps_v:
                wvh = [wv_pool.tile([128, 6 * 384], F16, tag=f"wv{h}",
                                    name=f"wv{h}") for h in range(2)]

                def wv(c, half):
                    return wvh[half][:, c * 384:(c + 1) * 384]

                # startup-critical loads, smallest-dependency-first:
                # wv(half0), xT(b0) -> C(b0,h0) can start; then the rest
                nc.sync.dma_start(
                    wvh[0][:].rearrange("p (c n) -> p c n", c=6),
                    bass.AP(qkvw_d.tensor, 2 * CDIM,
                            [[3 * CDIM, 128], [128 * 3 * CDIM, 6], [1, 384]]))
                nc.sync.dma_start(
                    xTb[0][:].rearrange("p (c n) -> p c n", c=6),
                    bass.AP(xT_d.tensor, 0,
                            [[NSEQ, 128], [128 * NSEQ, 6], [1, NTOK]]))
                nc.sync.dma_start(
                    wvh[1][:].rearrange("p (c n) -> p c n", c=6),
                    bass.AP(qkvw_d.tensor, 2 * CDIM + 384,
                            [[3 * CDIM, 128], [128 * 3 * CDIM, 6], [1, 384]]))
                nc.sync.dma_start(
                    xTb[1][:].rearrange("p (c n) -> p c n", c=6),
                    bass.AP(xT_d.tensor, NTOK,
                            [[NSEQ, 128], [128 * NSEQ, 6], [1, NTOK]]))

                # PE p-state warmup: junk matmuls during the input-DMA window
                # so the 3us clock ramp completes before real work arrives.
                warm = warmp.tile([128, 256], F16, tag="warm", name="warm")
                nc.vector.memset(warm[:], 0.0)
                psw = ps_w.tile([128, 256], F32, tag="psw", name="psw")
                for _ in range(30):
                    nc.tensor.matmul(psw[:], warm[:, 0:128], warm[:],
                                     start=True, stop=True)

                # ---- phase G: bias gather + exp/reorder per k-tile ----
                # SP DMA order: xT, wv, gather(t0), wqk, gather(t1..4), wot
                def _gather_t(t):
                    k0, pw = KTILES[t]
                    qlo = QLO[t]
                    bt = stagep.tile([128, WID[t] * NH], BF16, tag=f"bt{t}",
                                     name=f"bt{t}")
                    klo = max(k0, 1)
                    p0 = klo - k0
                    qg = max(qlo, 1)
                    colg = (qg - qlo) * NH
                    qw0 = (qg - 1) // GRID
                    QR = (NTOK - qg) // GRID
                    KR = (pw - p0) // GRID
                    assert (qg - 1) % GRID == 0 and (klo - 1) % GRID == 0
                    assert (NTOK - qg) % GRID == 0 and (pw - p0) % GRID == 0
                    for kr in range(KR):
                        kw = (klo - 1) // GRID + kr
                        d1_0 = qw0 - kw + (GRID - 1)
                        assert 0 <= d1_0 and d1_0 + QR <= 47
                        src = bass.AP(t3m_d.tensor, d1_0 * T3_D1,
                                      [[T3_KH, GRID], [T3_D1, QR], [1, T3_D1]])
                        dst = bt[p0 + kr * GRID:p0 + (kr + 1) * GRID,
                                 colg:colg + QR * T3_D1].rearrange(
                                     "p (qr i) -> p qr i", i=T3_D1)
                        nc.sync.dma_start(dst, src)
                    if t == 0:
                        # cls column k=0 (partition 0): constant, never masked
                        cnt = NTOK - qg
                        src = bass.AP(pe_d.tensor, (NRD - 2) * NH,
                                      [[0, 1], [0, cnt], [1, NH]])
                        dst = bt[0:1, colg:colg + cnt * NH].rearrange(
                            "p (a h) -> p a h", h=NH)
                        nc.gpsimd.dma_start(dst, src)  # casting DMA f32->bf16
                        # q=0 column: only (0,0) survives; its value cancels
                        # in the softmax normalization
                        nc.gpsimd.memset(bt[0:pw, 0:NH], NEG)
                        nc.gpsimd.memset(bt[0:1, 0:NH], 0.0)
                    # exp + (q,h)->(h,q) reorder on ACT
                    W = WID[t]
                    src = bt[0:pw, 0:W * NH].rearrange("p (q h) -> p q h", h=NH)
                    dst = expb[t][0:pw, :].rearrange("p (h q) -> p q h", h=NH)
                    nc.scalar.activation(dst, src,
                                         mybir.ActivationFunctionType.Exp)

                _gather_t(0)
                nc.sync.dma_start(
                    wqka[:].rearrange("p (c n) -> p c n", c=6),
                    bass.AP(qkvw_d.tensor, 0,
                            [[3 * CDIM, 128], [128 * 3 * CDIM, 6], [1, 1536]]))
                nc.sync.dma_start(
                    qkb[:], bass.AP(qkb_d.tensor, 0, [[1, 128], [128, 12]]))
                nc.sync.dma_start(
                    outb[:], bass.AP(outb_d.tensor, 0, [[1, 128], [128, 6]]))
                for t in range(1, 5):
                    _gather_t(t)
                # wot load last on SP before y stores
                nc.sync.dma_start(
                    wota[:].rearrange("p (c n) -> p c n", c=6),
                    bass.AP(outw_d.tensor, 0,
                            [[CDIM, 128], [128 * CDIM, 6], [1, CDIM]]))

                # ---- phase C: v (no bias; v-bias folded into out_b_eff) ----
                for b in range(BLOC):
                    for half in range(2):
                        for t, (k0, pw) in enumerate(KTILES):
                            vtile = vt[b][t]
                            ps = ps_v.tile([128, 384], F32, tag="psv", name="psv")
                            for c in range(6):
                                nc.tensor.matmul(
                                    ps[0:pw, :],
                                    xT(c, b)[:, k0:k0 + pw],
                                    wv(c, half),
                                    start=(c == 0), stop=(c == 5))
                            dst = vtile[0:pw, :].rearrange(
                                "p (h d) -> p h d",
                                h=NH)[:, half * 6:(half + 1) * 6, 0:64]
                            src = ps[0:pw, :].rearrange("p (h d) -> p h d", d=64)
                            nc.gpsimd.tensor_copy(dst, src)
                            if half == 1:
                                nc.gpsimd.memset(
                                    vtile[0:pw, :].rearrange(
                                        "p (h d) -> p h d",
                                        h=NH)[:, :, 64:128], 1.0)

            # ======== phases B + D interleaved per head-pair ========
            with tc.tile_pool(name="ps_qk", bufs=2, space="PSUM") as ps_qk, \
                 tc.tile_pool(name="ps_sT", bufs=4, space="PSUM") as ps_sT, \
                 tc.tile_pool(name="ps_OT", bufs=2, space="PSUM") as ps_OT, \
                 tc.tile_pool(name="es_pool", bufs=8) as es_pool, \
                 tc.tile_pool(name="p_pool", bufs=8) as p_pool:
                for jp in range(6):
                    # ---- B: produce qT[jp], kT[jp] ----
                    for r in (jp, jp + 6):
                        wcol0 = r * 128
                        dst = qT[r] if r < 6 else kT[r - 6]
                        for nb0, nbw in NBLK_B:
                            bb, loc = nb0 // NTOK, nb0 % NTOK
                            ps = ps_qk.tile([128, 386], F32, tag="psqk",
                                            name="psqk")
                            for c in range(6):
                                nc.tensor.matmul(ps[0:128, 0:nbw],
                                                 wqk(c)[:, wcol0:wcol0 + 128],
                                                 xT(c, bb)[:, loc:loc + nbw],
                                                 start=(c == 0), stop=(c == 5))
                            nc.vector.tensor_scalar_add(
                                dst[:, nb0:nb0 + nbw], ps[0:128, 0:nbw],
                                qkb[:, r:r + 1])
                    # ---- D: attention for both batches / both q-blocks ----
                    for b in range(BLOC):
                        for (qstart, qN) in QBLOCKS:
                            qend = qstart + qN
                            valid_t = [t for t in range(5) if QLO[t] < qend]
                            tlast = valid_t[-1]
                            psO = [ps_OT.tile([128, 456], F32, tag="psOT",
                                              name="psOT") for _ in range(2)]
                            # (att_tmp removed: denominator pre-replicated)
                            for t in valid_t:
                                k0, pw = KTILES[t]
                                qlo = max(qstart, QLO[t])
                                off = qlo - qstart
                                Nt = qend - qlo
                                ebase = qlo - QLO[t]
                                psS = [ps_sT.tile([128, 456], F32, tag="psS",
                                                  name="psS") for _ in range(2)]
                                for side in range(2):
                                    r0 = side * 64
                                    nc.tensor.matmul(
                                        psS[side][0:pw, 0:Nt],
                                        kT[jp][r0:r0 + 64,
                                               b * NTOK + k0:b * NTOK + k0 + pw],
                                        qT[jp][r0:r0 + 64,
                                               b * NTOK + qlo:b * NTOK + qlo + Nt],
                                        start=True, stop=True,
                                        tile_position=(r0, 0))
                                for side in range(2):
                                    h = 2 * jp + side
                                    es = es_pool.tile([128, 456], F16, tag="es",
                                                      name="es")
                                    nc.scalar.activation(
                                        es[0:pw, 0:Nt], psS[side][0:pw, 0:Nt],
                                        mybir.ActivationFunctionType.Exp)
                                    p = p_pool.tile([128, 456], F16, tag="p",
                                                    name="p")
                                    nc.vector.tensor_tensor(
                                        out=p[0:pw, 0:Nt],
                                        in0=es[0:pw, 0:Nt],
                                        in1=expb[t][0:pw,
                                                    h * WID[t] + ebase:
                                                    h * WID[t] + ebase + Nt],
                                        op=mybir.AluOpType.mult)
                                    nc.tensor.matmul(
                                        psO[side][0:128, off:off + Nt],
                                        vt[b][t][0:pw, h * 128:(h + 1) * 128],
                                        p[0:pw, 0:Nt],
                                        start=(t == valid_t[0]),
                                        stop=(t == tlast))
                            for side in range(2):
                                r0 = side * 64
                                nc.vector.tensor_tensor(
                                    out=oT[jp][r0:r0 + 64,
                                               b * NTOK + qstart:b * NTOK + qend],
                                    in0=psO[side][0:64, 0:qN],
                                    in1=psO[side][64:128, 0:qN],
                                    op=mybir.AluOpType.divide)

        # ================= phase E: output projection (yT layout) ============
        with tc.tile_pool(name="ps_o", bufs=3, space="PSUM") as ps_o, \
             tc.tile_pool(name="out_sb", bufs=1) as out_sb:
            ySB = [out_sb.tile([128, NSEQ], F16, tag=f"ySB{o}", name=f"ySB{o}")
                   for o in range(6)]
            for o in range(6):
                for nb0, nbw in NBLK:
                    ps = ps_o.tile([128, 386], F32, tag="pso", name="pso")
                    for c in range(6):
                        nc.tensor.matmul(
                            ps[0:128, 0:nbw],
                            wot(c)[:, o * 128:(o + 1) * 128],
                            oT[c][:, nb0:nb0 + nbw],
                            start=(c == 0), stop=(c == 5))
                    nc.scalar.activation(
                        ySB[o][:, nb0:nb0 + nbw], ps[0:128, 0:nbw],
                        mybir.ActivationFunctionType.Identity,
                        bias=outb[:, o:o + 1])
                    nc.sync.dma_start(
                        y_d[o * 128:(o + 1) * 128, nb0:nb0 + nbw],
                        ySB[o][:, nb0:nb0 + nbw])


def kernel(x, qkv_w, qkv_b, pos_emb, out_w, out_b, rel_index):
    x = np.asarray(x, dtype=np.float32)
    qkv_w = np.asarray(qkv_w, dtype=np.float32)
    qkv_b = np.asarray(qkv_b, dtype=np.float32)
    pos_emb = np.asarray(pos_emb, dtype=np.float32)
    out_w = np.asarray(out_w, dtype=np.float32)
    out_b = np.asarray(out_b, dtype=np.float32)
    ri = np.asarray(rel_index, dtype=np.int32)

    key = ri.tobytes()
    if key not in _CACHE:
        _CACHE[key] = _build(ri)
    nc = _CACHE[key]

    t3m, pos_embT = _host_prep(pos_emb, ri)
    # fold the attention scale into the q columns of qkv_w / qkv_b
    qkvw_h = qkv_w.astype(np.float16)
    qkvw_h[:, 0:CDIM] = (qkv_w[:, 0:CDIM] * SCALE).astype(np.float16)
    qkb_prep = qkv_b[0:2 * CDIM].astype(np.float32).copy()
    qkb_prep[0:CDIM] *= SCALE
    # v-bias commutes through the (normalized) attention into out projection
    outb_eff = (out_b + qkv_b[2 * CDIM:3 * CDIM] @ out_w).astype(np.float32)

    in_maps = []
    for c in range(NCORES):
        shard = np.ascontiguousarray(
            x[c * BLOC:(c + 1) * BLOC].reshape(NSEQ, CDIM).T.astype(np.float16))
        in_maps.append({
            "xT_in": shard,
            "qkv_w_h": qkvw_h,
            "qkb_prep": qkb_prep,
            "t3m": t3m,
            "pos_embT": pos_embT,
            "out_w_h": out_w.astype(np.float16),
            "out_b_eff": outb_eff,
        })
    res = run_bass_kernel_spmd(nc, in_maps, core_ids=list(range(NCORES)))
    out = np.empty((B, NTOK, CDIM), dtype=np.float32)
    for c in range(NCORES):
        yT = res.results[c]["yT"].astype(np.float32)     # [CDIM, NSEQ]
        out[c * BLOC:(c + 1) * BLOC] = yT.T.reshape(BLOC, NTOK, CDIM)
    return out


# revision 35
# speedup vs baseline: 1.1003x; 1.1003x over previous
"""Trainium2 Bass kernel for nn_EnhanceSelfAttention (B=16, N=577, C=768, H=12).

Self-contained: takes full unsharded inputs, shards batch across 8 NeuronCores
(2 batches/core), runs a fused attention kernel per core, gathers the output.

Per-core pipeline (f16 matmul operands, fp32 PSUM accumulation):
  Host prep: xT (pre-transposed f16 x shard), qkv_w f16 with the q-columns
  pre-scaled by d^-0.5, q-bias pre-scaled likewise, out_b folded with the
  v-bias contribution (attn rows sum to 1, so v-bias commutes to out_b via
  qkv_b[2C:] @ out_w), expanded bias table t3m (Toeplitz gather, causal mask
  folded as -65504).

  C. v = xT.T @ wv per causal k-tile, [k, 12*65] f16 with a ones column
     per head (softmax-denominator trick).  psum->sbuf casts on DVE.
  G. gather bias: 24 strided DMAs from t3m (1.1KB lines) -> exp+reorder on
     ACT into per-k-tile [k, h-major] f16 exp(bias) tiles (masked -> 0).
  B. qT,kT = wqk.T @ xT per head-pair, psum->sbuf + bias on DVE
     (tensor_scalar_add, scale pre-folded into weights).
  D. per head-pair: sT = kT.T@qT, both heads into one 2-bank psum tile;
     one fused exp (ACT) -> es[k, 2, q]; one fused mult with the 2-head
     expb slice (DVE, 2x mode); OT += v.T@p over causal k-tiles in PSUM.
     Row 64 = denominator; partition_broadcast + divide on GpSimd.
  E. yT = out_w.T @ oT (dim-major output), psum->sbuf + out_b_eff bias on
     ACT, f16 yT streamed to DRAM; host transposes back.
"""

import numpy as np
import ml_dtypes

import concourse.bass as bass
import concourse.tile as tile
from concourse import bacc, mybir
from concourse.bass_utils import run_bass_kernel_spmd

F32 = mybir.dt.float32
BF16 = mybir.dt.bfloat16
F16 = mybir.dt.float16

B, NTOK, CDIM, NH, DH = 16, 577, 768, 12, 64
GRID = 24
NRD = (2 * GRID - 1) * (2 * GRID - 1) + 3  # 2212
NCORES = 8
BLOC = B // NCORES       # batches per core
NSEQ = BLOC * NTOK       # 1154
SCALE = DH ** -0.5       # 0.125
NEG = -65504.0

QBLOCKS = [(0, 121), (121, 456)]            # (qstart, qN)
# k-tiles: (k0, pw).  t=0: partitions 0..120 <-> k=0..120 (incl cls col k=0)
KTILES = [(0, 121), (121, 120), (241, 120), (361, 120), (481, 96)]
# per-tile stored q range: [QLO[t] .. 577)
QLO = [0, 121, 241, 361, 481]
WID = [NTOK - q for q in QLO]               # 577, 456, 336, 216, 96

# expanded-table strides (T3m[kh, d1, qh, h], d1 = qw-kw+23)
T3_KH = 47 * GRID * NH   # 13536
T3_D1 = GRID * NH        # 288

_CACHE = {}


def _check_rel_index(ri):
    """Assert the Toeplitz structure the gather DMAs rely on."""
    assert ri.shape == (NTOK, NTOK)
    assert ri[0, 0] == NRD - 1
    assert (ri[0, 1:] == NRD - 3).all()
    assert (ri[1:, 0] == NRD - 2).all()
    a = np.arange(NTOK - 1)
    qh, qw = a % GRID, a // GRID
    rel0 = qh[:, None] - qh[None, :] + GRID - 1
    rel1 = qw[:, None] - qw[None, :] + GRID - 1
    expect = rel0 + rel1 * (2 * GRID - 1)
    assert np.array_equal(ri[1:, 1:], expect), "rel_index lacks expected structure"


def _host_prep(pos_emb, rel_index):
    _check_rel_index(np.asarray(rel_index))
    pe_t = np.asarray(pos_emb, dtype=np.float32).T      # [NRD, NH]
    # expanded gather table with the causal mask folded in:
    # T3m[kh, d1, qh, h] = pos_emb[h, (qh-kh+23) + 47*d1] + (NEG if q<k)
    # where q-k = (qh-kh) + 24*(d1-23)
    kh = np.arange(GRID)[:, None, None]
    d1 = np.arange(2 * GRID - 1)[None, :, None]
    qh = np.arange(GRID)[None, None, :]
    ridx = (qh - kh + GRID - 1) + (2 * GRID - 1) * d1   # [24, 47, 24]
    t3m = pe_t[ridx]                                    # [24, 47, 24, NH]
    masked = (qh - kh) + GRID * (d1 - (GRID - 1)) < 0
    t3m = t3m + np.where(masked, NEG, 0.0)[..., None].astype(np.float32)
    t3m = np.ascontiguousarray(t3m.reshape(-1)).astype(ml_dtypes.bfloat16)
    pos_embT = np.ascontiguousarray(pe_t).reshape(-1)
    return t3m, pos_embT


def _build(ri):
    """Build + compile the per-core Bass program."""
    nc = bacc.Bacc("TRN2", target_bir_lowering=False, debug=False)

    xT_d = nc.dram_tensor("xT_in", [CDIM, NSEQ], F16, kind="ExternalInput").ap()
    qkvwh_d = nc.dram_tensor("qkv_w_h", [CDIM, 3 * CDIM], F16,
                             kind="ExternalInput").ap()
    qkb_d = nc.dram_tensor("qkb_prep", [2 * CDIM], F32,
                           kind="ExternalInput").ap()
    t3m_d = nc.dram_tensor("t3m", [GRID * 47 * GRID * NH], BF16,
                           kind="ExternalInput").ap()
    pe_d = nc.dram_tensor("pos_embT", [NRD * NH], F32, kind="ExternalInput").ap()
    outwh_d = nc.dram_tensor("out_w_h", [CDIM, CDIM], F16,
                             kind="ExternalInput").ap()
    outb_d = nc.dram_tensor("out_b_eff", [CDIM], F32, kind="ExternalInput").ap()
    y_d = nc.dram_tensor("yT", [CDIM, NSEQ], F16, kind="ExternalOutput").ap()

    with tile.TileContext(nc) as tc:
        _emit(nc, tc, xT_d, qkvwh_d, qkb_d, t3m_d, pe_d, outwh_d, outb_d, y_d)
    nc.compile()
    return nc


def _emit(nc, tc, xT_d, qkvw_d, qkb_d, t3m_d, pe_d, outw_d, outb_d, y_d):
    from contextlib import ExitStack

    NBLK = [(0, 386), (386, 384), (770, 384)]   # n-blocks (E out proj)
    # B blocks aligned to the batch boundary (577) so each reads one xT tile
    NBLK_B = [(0, 289), (289, 288), (577, 289), (866, 288)]

    with ExitStack() as top:
        persist = top.enter_context(tc.tile_pool(name="persist", bufs=1))
        consts = top.enter_context(tc.tile_pool(name="consts", bufs=1))
        wo_pool = top.enter_context(tc.tile_pool(name="wo", bufs=1))

        # ---- constants (tiny DMAs, SP queue, issued after wqk) ----
        # qk bias columns packed [128, 12] (host pre-scaled the q half)
        qkb = consts.tile([128, 12], F32, tag="qkb", name="qkb")
        # out_b_eff columns packed [128, 6]
        outb = consts.tile([128, 6], F32, tag="outb", name="outb")

        # ---- persistent activation storage (all f16) ----
        qT = [persist.tile([128, NSEQ], F16, tag=f"qT{j}", name=f"qT{j}")
              for j in range(6)]
        kT = [persist.tile([128, NSEQ], F16, tag=f"kT{j}", name=f"kT{j}")
              for j in range(6)]
        # per head: 64 value cols + 64 ones cols -> AV rows 64:128 hold the
        # softmax denominator already replicated across partitions (no
        # partition_broadcast needed before the divide)
        vt = [[persist.tile([128, NH * 128], F16, tag=f"v{b}_{t}",
                            name=f"v{b}_{t}")
               for t in range(5)] for b in range(BLOC)]
        oT = [persist.tile([128, NSEQ], F16, tag=f"oT{j}", name=f"oT{j}")
              for j in range(6)]
        expb = [persist.tile([128, WID[t] * NH], F16, tag=f"expb{t}",
                             name=f"expb{t}") for t in range(5)]
        wota = wo_pool.tile([128, 6 * CDIM], F16, tag="wota", name="wota")

        def wot(c):
            return wota[:, c * CDIM:(c + 1) * CDIM]

        # ================= phase 1: projections + gather =================
        with tc.tile_pool(name="xT", bufs=1) as xTp, \
             tc.tile_pool(name="wqk_pool", bufs=1) as wqk_pool, \
             tc.tile_pool(name="stage", bufs=1) as stagep:
            # all 6 row-chunks side by side per tile; xT split per batch and
            # wv per column-half so phase C's first groups depend on the
            # smallest possible DMA set
            xTb = [xTp.tile([128, 6 * NTOK], F16, tag=f"xT{b}", name=f"xT{b}")
                   for b in range(BLOC)]
            wqka = wqk_pool.tile([128, 6 * 1536], F16, tag="wqka", name="wqka")

            def xT(c, b):
                return xTb[b][:, c * NTOK:(c + 1) * NTOK]

            def wqk(c):
                return wqka[:, c * 1536:(c + 1) * 1536]

            # DMA order on SP queue: wv -> xT(b0) -> xT(b1) -> gathers ->
            # wqk -> consts -> wot.  xT split per batch so C(b0) starts early.
            with tc.tile_pool(name="wv_pool", bufs=1) as wv_pool, \
                 tc.tile_pool(name="warm", bufs=1) as warmp, \
                 tc.tile_pool(name="ps_w", bufs=1, space="PSUM") as ps_w, \
                 tc.tile_pool(name="ps_v", bufs=5, space="PSUM") as ps_v:
                wvh = [wv_pool.tile([128, 6 * 384], F16, tag=f"wv{h}",
                                    name=f"wv{h}") for h in range(2)]

                def wv(c, half):
                    return wvh[half][:, c * 384:(c + 1) * 384]

                # startup-critical loads, smallest-dependency-first:
                # wv(half0), xT(b0) -> C(b0,h0) can start; then the rest
                nc.sync.dma_start(
                    wvh[0][:].rearrange("p (c n) -> p c n", c=6),
                    bass.AP(qkvw_d.tensor, 2 * CDIM,
                            [[3 * CDIM, 128], [128 * 3 * CDIM, 6], [1, 384]]))
                nc.sync.dma_start(
                    xTb[0][:].rearrange("p (c n) -> p c n", c=6),
                    bass.AP(xT_d.tensor, 0,
                            [[NSEQ, 128], [128 * NSEQ, 6], [1, NTOK]]))
                nc.sync.dma_start(
                    wvh[1][:].rearrange("p (c n) -> p c n", c=6),
                    bass.AP(qkvw_d.tensor, 2 * CDIM + 384,
                            [[3 * CDIM, 128], [128 * 3 * CDIM, 6], [1, 384]]))
                nc.sync.dma_start(
                    xTb[1][:].rearrange("p (c n) -> p c n", c=6),
                    bass.AP(xT_d.tensor, NTOK,
                            [[NSEQ, 128], [128 * NSEQ, 6], [1, NTOK]]))

                # PE p-state warmup: junk matmuls during the input-DMA window
                # so the 3us clock ramp completes before real work arrives.
                warm = warmp.tile([128, 256], F16, tag="warm", name="warm")
                nc.vector.memset(warm[:], 0.0)
                psw = ps_w.tile([128, 256], F32, tag="psw", name="psw")
                for _ in range(44):
                    nc.tensor.matmul(psw[:], warm[:, 0:128], warm[:],
                                     start=True, stop=True)

                # ---- phase G: bias gather + exp/reorder per k-tile ----
                # SP DMA order: xT, wv, gather(t0), wqk, gather(t1..4), wot
                def _gather_t(t):
                    k0, pw = KTILES[t]
                    qlo = QLO[t]
                    bt = stagep.tile([128, WID[t] * NH], BF16, tag=f"bt{t}",
                                     name=f"bt{t}")
                    klo = max(k0, 1)
                    p0 = klo - k0
                    qg = max(qlo, 1)
                    colg = (qg - qlo) * NH
                    qw0 = (qg - 1) // GRID
                    QR = (NTOK - qg) // GRID
                    KR = (pw - p0) // GRID
                    assert (qg - 1) % GRID == 0 and (klo - 1) % GRID == 0
                    assert (NTOK - qg) % GRID == 0 and (pw - p0) % GRID == 0
                    for kr in range(KR):
                        kw = (klo - 1) // GRID + kr
                        d1_0 = qw0 - kw + (GRID - 1)
                        assert 0 <= d1_0 and d1_0 + QR <= 47
                        src = bass.AP(t3m_d.tensor, d1_0 * T3_D1,
                                      [[T3_KH, GRID], [T3_D1, QR], [1, T3_D1]])
                        dst = bt[p0 + kr * GRID:p0 + (kr + 1) * GRID,
                                 colg:colg + QR * T3_D1].rearrange(
                                     "p (qr i) -> p qr i", i=T3_D1)
                        nc.sync.dma_start(dst, src)
                    if t == 0:
                        # cls column k=0 (partition 0): constant, never masked
                        cnt = NTOK - qg
                        src = bass.AP(pe_d.tensor, (NRD - 2) * NH,
                                      [[0, 1], [0, cnt], [1, NH]])
                        dst = bt[0:1, colg:colg + cnt * NH].rearrange(
                            "p (a h) -> p a h", h=NH)
                        nc.gpsimd.dma_start(dst, src)  # casting DMA f32->bf16
                        # q=0 column: only (0,0) survives; its value cancels
                        # in the softmax normalization
                        nc.gpsimd.memset(bt[0:pw, 0:NH], NEG)
                        nc.gpsimd.memset(bt[0:1, 0:NH], 0.0)
                    # exp + (q,h)->(h,q) reorder on ACT
                    W = WID[t]
                    src = bt[0:pw, 0:W * NH].rearrange("p (q h) -> p q h", h=NH)
                    dst = expb[t][0:pw, :].rearrange("p (h q) -> p q h", h=NH)
                    nc.scalar.activation(dst, src,
                                         mybir.ActivationFunctionType.Exp)

                _gather_t(0)
                _gather_t(1)
                nc.sync.dma_start(
                    wqka[:].rearrange("p (c n) -> p c n", c=6),
                    bass.AP(qkvw_d.tensor, 0,
                            [[3 * CDIM, 128], [128 * 3 * CDIM, 6], [1, 1536]]))
                nc.sync.dma_start(
                    qkb[:], bass.AP(qkb_d.tensor, 0, [[1, 128], [128, 12]]))
                nc.sync.dma_start(
                    outb[:], bass.AP(outb_d.tensor, 0, [[1, 128], [128, 6]]))
                for t in range(2, 5):
                    _gather_t(t)
                # wot load last on SP before y stores
                nc.sync.dma_start(
                    wota[:].rearrange("p (c n) -> p c n", c=6),
                    bass.AP(outw_d.tensor, 0,
                            [[CDIM, 128], [128 * CDIM, 6], [1, CDIM]]))

                # ---- phase C: v (no bias; v-bias folded into out_b_eff) ----
                for b in range(BLOC):
                    for half in range(2):
                        for t, (k0, pw) in enumerate(KTILES):
                            vtile = vt[b][t]
                            ps = ps_v.tile([128, 384], F32, tag="psv", name="psv")
                            for c in range(6):
                                nc.tensor.matmul(
                                    ps[0:pw, :],
                                    xT(c, b)[:, k0:k0 + pw],
                                    wv(c, half),
                                    start=(c == 0), stop=(c == 5))
                            dst = vtile[0:pw, :].rearrange(
                                "p (h d) -> p h d",
                                h=NH)[:, half * 6:(half + 1) * 6, 0:64]
                            src = ps[0:pw, :].rearrange("p (h d) -> p h d", d=64)
                            nc.gpsimd.tensor_copy(dst, src)
                            if half == 1:
                                nc.gpsimd.memset(
                                    vtile[0:pw, :].rearrange(
                                        "p (h d) -> p h d",
                                        h=NH)[:, :, 64:128], 1.0)

            # ======== phases B + D interleaved per head-pair ========
            with tc.tile_pool(name="ps_qk", bufs=2, space="PSUM") as ps_qk, \
                 tc.tile_pool(name="ps_sT", bufs=4, space="PSUM") as ps_sT, \
                 tc.tile_pool(name="ps_OT", bufs=2, space="PSUM") as ps_OT, \
                 tc.tile_pool(name="es_pool", bufs=8) as es_pool, \
                 tc.tile_pool(name="p_pool", bufs=8) as p_pool:
                for jp in range(6):
                    # ---- B: produce qT[jp], kT[jp] ----
                    for r in (jp, jp + 6):
                        wcol0 = r * 128
                        dst = qT[r] if r < 6 else kT[r - 6]
                        for nb0, nbw in NBLK_B:
                            bb, loc = nb0 // NTOK, nb0 % NTOK
                            ps = ps_qk.tile([128, 386], F32, tag="psqk",
                                            name="psqk")
                            for c in range(6):
                                nc.tensor.matmul(ps[0:128, 0:nbw],
                                                 wqk(c)[:, wcol0:wcol0 + 128],
                                                 xT(c, bb)[:, loc:loc + nbw],
                                                 start=(c == 0), stop=(c == 5))
                            nc.vector.tensor_scalar_add(
                                dst[:, nb0:nb0 + nbw], ps[0:128, 0:nbw],
                                qkb[:, r:r + 1])
                    # ---- D: attention for both batches / both q-blocks ----
                    for b in range(BLOC):
                        for (qstart, qN) in QBLOCKS:
                            qend = qstart + qN
                            valid_t = [t for t in range(5) if QLO[t] < qend]
                            tlast = valid_t[-1]
                            psO = [ps_OT.tile([128, 456], F32, tag="psOT",
                                              name="psOT") for _ in range(2)]
                            # (att_tmp removed: denominator pre-replicated)
                            for t in valid_t:
                                k0, pw = KTILES[t]
                                qlo = max(qstart, QLO[t])
                                off = qlo - qstart
                                Nt = qend - qlo
                                ebase = qlo - QLO[t]
                                psS = [ps_sT.tile([128, 456], F32, tag="psS",
                                                  name="psS") for _ in range(2)]
                                for side in range(2):
                                    r0 = side * 64
                                    nc.tensor.matmul(
                                        psS[side][0:pw, 0:Nt],
                                        kT[jp][r0:r0 + 64,
                                               b * NTOK + k0:b * NTOK + k0 + pw],
                                        qT[jp][r0:r0 + 64,
                                               b * NTOK + qlo:b * NTOK + qlo + Nt],
                                        start=True, stop=True,
                                        tile_position=(r0, 0))
                                for side in range(2):
                                    h = 2 * jp + side
                                    es = es_pool.tile([128, 456], F16, tag="es",
                                                      name="es")
                                    nc.scalar.activation(
                                        es[0:pw, 0:Nt], psS[side][0:pw, 0:Nt],
                                        mybir.ActivationFunctionType.Exp)
                                    p = p_pool.tile([128, 456], F16, tag="p",
                                                    name="p")
                                    nc.vector.tensor_tensor(
                                        out=p[0:pw, 0:Nt],
                                        in0=es[0:pw, 0:Nt],
                                        in1=expb[t][0:pw,
                                                    h * WID[t] + ebase:
                                                    h * WID[t] + ebase + Nt],
                                        op=mybir.AluOpType.mult)
                                    nc.tensor.matmul(
                                        psO[side][0:128, off:off + Nt],
                                        vt[b][t][0:pw, h * 128:(h + 1) * 128],
                                        p[0:pw, 0:Nt],
                                        start=(t == valid_t[0]),
                                        stop=(t == tlast))
                            for side in range(2):
                                r0 = side * 64
                                nc.vector.tensor_tensor(
                                    out=oT[jp][r0:r0 + 64,
                                               b * NTOK + qstart:b * NTOK + qend],
                                    in0=psO[side][0:64, 0:qN],
                                    in1=psO[side][64:128, 0:qN],
                                    op=mybir.AluOpType.divide)

        # ================= phase E: output projection (yT layout) ============
        with tc.tile_pool(name="ps_o", bufs=3, space="PSUM") as ps_o, \
             tc.tile_pool(name="out_sb", bufs=1) as out_sb:
            ySB = [out_sb.tile([128, NSEQ], F16, tag=f"ySB{o}", name=f"ySB{o}")
                   for o in range(6)]
            for o in range(6):
                for nb0, nbw in NBLK:
                    ps = ps_o.tile([128, 386], F32, tag="pso", name="pso")
                    for c in range(6):
                        nc.tensor.matmul(
                            ps[0:128, 0:nbw],
                            wot(c)[:, o * 128:(o + 1) * 128],
                            oT[c][:, nb0:nb0 + nbw],
                            start=(c == 0), stop=(c == 5))
                    nc.scalar.activation(
                        ySB[o][:, nb0:nb0 + nbw], ps[0:128, 0:nbw],
                        mybir.ActivationFunctionType.Identity,
                        bias=outb[:, o:o + 1])
                    nc.sync.dma_start(
                        y_d[o * 128:(o + 1) * 128, nb0:nb0 + nbw],
                        ySB[o][:, nb0:nb0 + nbw])


def kernel(x, qkv_w, qkv_b, pos_emb, out_w, out_b, rel_index):
    x = np.asarray(x, dtype=np.float32)
    qkv_w = np.asarray(qkv_w, dtype=np.float32)
    qkv_b = np.asarray(qkv_b, dtype=np.float32)
    pos_emb = np.asarray(pos_emb, dtype=np.float32)
    out_w = np.asarray(out_w, dtype=np.float32)
    out_b = np.asarray(out_b, dtype=np.float32)
    ri = np.asarray(rel_index, dtype=np.int32)

    key = ri.tobytes()
    if key not in _CACHE:
        _CACHE[key] = _build(ri)
    nc = _CACHE[key]

    t3m, pos_embT = _host_prep(pos_emb, ri)
    # fold the attention scale into the q columns of qkv_w / qkv_b
    qkvw_h = qkv_w.astype(np.float16)
    qkvw_h[:, 0:CDIM] = (qkv_w[:, 0:CDIM] * SCALE).astype(np.float16)
    qkb_prep = qkv_b[0:2 * CDIM].astype(np.float32).copy()
    qkb_prep[0:CDIM] *= SCALE
    # v-bias commutes through the (normalized) attention into out projection
    outb_eff = (out_b + qkv_b[2 * CDIM:3 * CDIM] @ out_w).astype(np.float32)

    in_maps = []
    for c in range(NCORES):
        shard = np.ascontiguousarray(
            x[c * BLOC:(c + 1) * BLOC].reshape(NSEQ, CDIM).T.astype(np.float16))
        in_maps.append({
            "xT_in": shard,
            "qkv_w_h": qkvw_h,
            "qkb_prep": qkb_prep,
            "t3m": t3m,
            "pos_embT": pos_embT,
            "out_w_h": out_w.astype(np.float16),
            "out_b_eff": outb_eff,
        })
    res = run_bass_kernel_spmd(nc, in_maps, core_ids=list(range(NCORES)))
    out = np.empty((B, NTOK, CDIM), dtype=np.float32)
    for c in range(NCORES):
        yT = res.results[c]["yT"].astype(np.float32)     # [CDIM, NSEQ]
        out[c * BLOC:(c + 1) * BLOC] = yT.T.reshape(BLOC, NTOK, CDIM)
    return out


# revision 36
# speedup vs baseline: 1.1100x; 1.0089x over previous
"""Trainium2 Bass kernel for nn_EnhanceSelfAttention (B=16, N=577, C=768, H=12).

Self-contained: takes full unsharded inputs, shards batch across 8 NeuronCores
(2 batches/core), runs a fused attention kernel per core, gathers the output.

Per-core pipeline (f16 matmul operands, fp32 PSUM accumulation):
  Host prep: xT (pre-transposed f16 x shard), qkv_w f16 with the q-columns
  pre-scaled by d^-0.5, q-bias pre-scaled likewise, out_b folded with the
  v-bias contribution (attn rows sum to 1, so v-bias commutes to out_b via
  qkv_b[2C:] @ out_w), expanded bias table t3m (Toeplitz gather, causal mask
  folded as -65504).

  C. v = xT.T @ wv per causal k-tile, [k, 12*65] f16 with a ones column
     per head (softmax-denominator trick).  psum->sbuf casts on DVE.
  G. gather bias: 24 strided DMAs from t3m (1.1KB lines) -> exp+reorder on
     ACT into per-k-tile [k, h-major] f16 exp(bias) tiles (masked -> 0).
  B. qT,kT = wqk.T @ xT per head-pair, psum->sbuf + bias on DVE
     (tensor_scalar_add, scale pre-folded into weights).
  D. per head-pair: sT = kT.T@qT, both heads into one 2-bank psum tile;
     one fused exp (ACT) -> es[k, 2, q]; one fused mult with the 2-head
     expb slice (DVE, 2x mode); OT += v.T@p over causal k-tiles in PSUM.
     Row 64 = denominator; partition_broadcast + divide on GpSimd.
  E. yT = out_w.T @ oT (dim-major output), psum->sbuf + out_b_eff bias on
     ACT, f16 yT streamed to DRAM; host transposes back.
"""

import numpy as np
import ml_dtypes

import concourse.bass as bass
import concourse.tile as tile
from concourse import bacc, mybir
from concourse.bass_utils import run_bass_kernel_spmd

F32 = mybir.dt.float32
BF16 = mybir.dt.bfloat16
F16 = mybir.dt.float16

B, NTOK, CDIM, NH, DH = 16, 577, 768, 12, 64
GRID = 24
NRD = (2 * GRID - 1) * (2 * GRID - 1) + 3  # 2212
NCORES = 8
BLOC = B // NCORES       # batches per core
NSEQ = BLOC * NTOK       # 1154
SCALE = DH ** -0.5       # 0.125
NEG = -65504.0

QBLOCKS = [(0, 121), (121, 456)]            # (qstart, qN)
# k-tiles: (k0, pw).  t=0: partitions 0..120 <-> k=0..120 (incl cls col k=0)
KTILES = [(0, 121), (121, 120), (241, 120), (361, 120), (481, 96)]
# per-tile stored q range: [QLO[t] .. 577)
QLO = [0, 121, 241, 361, 481]
WID = [NTOK - q for q in QLO]               # 577, 456, 336, 216, 96

# expanded-table strides (T3m[kh, d1, qh, h], d1 = qw-kw+23)
T3_KH = 47 * GRID * NH   # 13536
T3_D1 = GRID * NH        # 288

_CACHE = {}


def _check_rel_index(ri):
    """Assert the Toeplitz structure the gather DMAs rely on."""
    assert ri.shape == (NTOK, NTOK)
    assert ri[0, 0] == NRD - 1
    assert (ri[0, 1:] == NRD - 3).all()
    assert (ri[1:, 0] == NRD - 2).all()
    a = np.arange(NTOK - 1)
    qh, qw = a % GRID, a // GRID
    rel0 = qh[:, None] - qh[None, :] + GRID - 1
    rel1 = qw[:, None] - qw[None, :] + GRID - 1
    expect = rel0 + rel1 * (2 * GRID - 1)
    assert np.array_equal(ri[1:, 1:], expect), "rel_index lacks expected structure"


def _host_prep(pos_emb, rel_index):
    _check_rel_index(np.asarray(rel_index))
    pe_t = np.asarray(pos_emb, dtype=np.float32).T      # [NRD, NH]
    # expanded gather table with the causal mask folded in:
    # T3m[kh, d1, qh, h] = pos_emb[h, (qh-kh+23) + 47*d1] + (NEG if q<k)
    # where q-k = (qh-kh) + 24*(d1-23)
    kh = np.arange(GRID)[:, None, None]
    d1 = np.arange(2 * GRID - 1)[None, :, None]
    qh = np.arange(GRID)[None, None, :]
    ridx = (qh - kh + GRID - 1) + (2 * GRID - 1) * d1   # [24, 47, 24]
    t3m = pe_t[ridx]                                    # [24, 47, 24, NH]
    masked = (qh - kh) + GRID * (d1 - (GRID - 1)) < 0
    t3m = t3m + np.where(masked, NEG, 0.0)[..., None].astype(np.float32)
    t3m = np.ascontiguousarray(t3m.reshape(-1)).astype(ml_dtypes.bfloat16)
    pos_embT = np.ascontiguousarray(pe_t).reshape(-1)
    return t3m, pos_embT


def _build(ri):
    """Build + compile the per-core Bass program."""
    nc = bacc.Bacc("TRN2", target_bir_lowering=False, debug=False)

    xT_d = nc.dram_tensor("xT_in", [CDIM, NSEQ], F16, kind="ExternalInput").ap()
    qkvwh_d = nc.dram_tensor("qkv_w_h", [CDIM, 3 * CDIM], F16,
                             kind="ExternalInput").ap()
    qkb_d = nc.dram_tensor("qkb_prep", [2 * CDIM], F32,
                           kind="ExternalInput").ap()
    t3m_d = nc.dram_tensor("t3m", [GRID * 47 * GRID * NH], BF16,
                           kind="ExternalInput").ap()
    pe_d = nc.dram_tensor("pos_embT", [NRD * NH], F32, kind="ExternalInput").ap()
    outwh_d = nc.dram_tensor("out_w_h", [CDIM, CDIM], F16,
                             kind="ExternalInput").ap()
    outb_d = nc.dram_tensor("out_b_eff", [CDIM], F32, kind="ExternalInput").ap()
    y_d = nc.dram_tensor("yT", [CDIM, NSEQ], F16, kind="ExternalOutput").ap()

    with tile.TileContext(nc) as tc:
        _emit(nc, tc, xT_d, qkvwh_d, qkb_d, t3m_d, pe_d, outwh_d, outb_d, y_d)
    nc.compile()
    return nc


def _emit(nc, tc, xT_d, qkvw_d, qkb_d, t3m_d, pe_d, outw_d, outb_d, y_d):
    from contextlib import ExitStack

    NBLK = [(0, 386), (386, 384), (770, 384)]   # n-blocks (E out proj)
    # B blocks aligned to the batch boundary (577) so each reads one xT tile
    NBLK_B = [(0, 289), (289, 288), (577, 289), (866, 288)]

    with ExitStack() as top:
        persist = top.enter_context(tc.tile_pool(name="persist", bufs=1))
        consts = top.enter_context(tc.tile_pool(name="consts", bufs=1))
        wo_pool = top.enter_context(tc.tile_pool(name="wo", bufs=1))

        # ---- constants (tiny DMAs, SP queue, issued after wqk) ----
        # qk bias columns packed [128, 12] (host pre-scaled the q half)
        qkb = consts.tile([128, 12], F32, tag="qkb", name="qkb")
        # out_b_eff columns packed [128, 6]
        outb = consts.tile([128, 6], F32, tag="outb", name="outb")

        # ---- persistent activation storage (all f16) ----
        qT = [persist.tile([128, NSEQ], F16, tag=f"qT{j}", name=f"qT{j}")
              for j in range(6)]
        kT = [persist.tile([128, NSEQ], F16, tag=f"kT{j}", name=f"kT{j}")
              for j in range(6)]
        # per head: 64 value cols + 64 ones cols -> AV rows 64:128 hold the
        # softmax denominator already replicated across partitions (no
        # partition_broadcast needed before the divide)
        vt = [[persist.tile([128, NH * 128], F16, tag=f"v{b}_{t}",
                            name=f"v{b}_{t}")
               for t in range(5)] for b in range(BLOC)]
        oT = [persist.tile([128, NSEQ], F16, tag=f"oT{j}", name=f"oT{j}")
              for j in range(6)]
        expb = [persist.tile([128, WID[t] * NH], F16, tag=f"expb{t}",
                             name=f"expb{t}") for t in range(5)]
        wota = wo_pool.tile([128, 6 * CDIM], F16, tag="wota", name="wota")

        def wot(c):
            return wota[:, c * CDIM:(c + 1) * CDIM]

        # ================= phase 1: projections + gather =================
        with tc.tile_pool(name="xT", bufs=1) as xTp, \
             tc.tile_pool(name="wqk_pool", bufs=1) as wqk_pool, \
             tc.tile_pool(name="stage", bufs=1) as stagep:
            # all 6 row-chunks side by side per tile; xT split per batch and
            # wv per column-half so phase C's first groups depend on the
            # smallest possible DMA set
            xTb = [xTp.tile([128, 6 * NTOK], F16, tag=f"xT{b}", name=f"xT{b}")
                   for b in range(BLOC)]
            wqka = wqk_pool.tile([128, 6 * 1536], F16, tag="wqka", name="wqka")

            def xT(c, b):
                return xTb[b][:, c * NTOK:(c + 1) * NTOK]

            def wqk(c):
                return wqka[:, c * 1536:(c + 1) * 1536]

            # DMA order on SP queue: wv -> xT(b0) -> xT(b1) -> gathers ->
            # wqk -> consts -> wot.  xT split per batch so C(b0) starts early.
            with tc.tile_pool(name="wv_pool", bufs=1) as wv_pool, \
                 tc.tile_pool(name="warm", bufs=1) as warmp, \
                 tc.tile_pool(name="ps_w", bufs=1, space="PSUM") as ps_w, \
                 tc.tile_pool(name="ps_v", bufs=5, space="PSUM") as ps_v:
                wvh = [wv_pool.tile([128, 6 * 384], F16, tag=f"wv{h}",
                                    name=f"wv{h}") for h in range(2)]

                def wv(c, half):
                    return wvh[half][:, c * 384:(c + 1) * 384]

                # startup-critical loads, smallest-dependency-first:
                # wv(half0), xT(b0) -> C(b0,h0) can start; then the rest
                nc.sync.dma_start(
                    wvh[0][:].rearrange("p (c n) -> p c n", c=6),
                    bass.AP(qkvw_d.tensor, 2 * CDIM,
                            [[3 * CDIM, 128], [128 * 3 * CDIM, 6], [1, 384]]))
                nc.sync.dma_start(
                    xTb[0][:].rearrange("p (c n) -> p c n", c=6),
                    bass.AP(xT_d.tensor, 0,
                            [[NSEQ, 128], [128 * NSEQ, 6], [1, NTOK]]))
                nc.sync.dma_start(
                    wvh[1][:].rearrange("p (c n) -> p c n", c=6),
                    bass.AP(qkvw_d.tensor, 2 * CDIM + 384,
                            [[3 * CDIM, 128], [128 * 3 * CDIM, 6], [1, 384]]))
                nc.sync.dma_start(
                    xTb[1][:].rearrange("p (c n) -> p c n", c=6),
                    bass.AP(xT_d.tensor, NTOK,
                            [[NSEQ, 128], [128 * NSEQ, 6], [1, NTOK]]))

                # PE p-state warmup: junk matmuls during the input-DMA window
                # so the 3us clock ramp completes before real work arrives.
                warm = warmp.tile([128, 256], F16, tag="warm", name="warm")
                nc.vector.memset(warm[:], 0.0)
                psw = ps_w.tile([128, 256], F32, tag="psw", name="psw")
                for _ in range(28):
                    nc.tensor.matmul(psw[:], warm[:, 0:128], warm[:],
                                     start=True, stop=True)

                # ---- phase G: bias gather + exp/reorder per k-tile ----
                # SP DMA order: xT, wv, gather(t0), wqk, gather(t1..4), wot
                def _gather_t(t):
                    k0, pw = KTILES[t]
                    qlo = QLO[t]
                    bt = stagep.tile([128, WID[t] * NH], BF16, tag=f"bt{t}",
                                     name=f"bt{t}")
                    klo = max(k0, 1)
                    p0 = klo - k0
                    qg = max(qlo, 1)
                    colg = (qg - qlo) * NH
                    qw0 = (qg - 1) // GRID
                    QR = (NTOK - qg) // GRID
                    KR = (pw - p0) // GRID
                    assert (qg - 1) % GRID == 0 and (klo - 1) % GRID == 0
                    assert (NTOK - qg) % GRID == 0 and (pw - p0) % GRID == 0
                    for kr in range(KR):
                        kw = (klo - 1) // GRID + kr
                        d1_0 = qw0 - kw + (GRID - 1)
                        assert 0 <= d1_0 and d1_0 + QR <= 47
                        src = bass.AP(t3m_d.tensor, d1_0 * T3_D1,
                                      [[T3_KH, GRID], [T3_D1, QR], [1, T3_D1]])
                        dst = bt[p0 + kr * GRID:p0 + (kr + 1) * GRID,
                                 colg:colg + QR * T3_D1].rearrange(
                                     "p (qr i) -> p qr i", i=T3_D1)
                        nc.sync.dma_start(dst, src)
                    if t == 0:
                        # cls column k=0 (partition 0): constant, never masked
                        cnt = NTOK - qg
                        src = bass.AP(pe_d.tensor, (NRD - 2) * NH,
                                      [[0, 1], [0, cnt], [1, NH]])
                        dst = bt[0:1, colg:colg + cnt * NH].rearrange(
                            "p (a h) -> p a h", h=NH)
                        nc.gpsimd.dma_start(dst, src)  # casting DMA f32->bf16
                        # q=0 column: only (0,0) survives; its value cancels
                        # in the softmax normalization
                        nc.gpsimd.memset(bt[0:pw, 0:NH], NEG)
                        nc.gpsimd.memset(bt[0:1, 0:NH], 0.0)
                    # exp + (q,h)->(h,q) reorder on ACT
                    W = WID[t]
                    src = bt[0:pw, 0:W * NH].rearrange("p (q h) -> p q h", h=NH)
                    dst = expb[t][0:pw, :].rearrange("p (h q) -> p q h", h=NH)
                    nc.scalar.activation(dst, src,
                                         mybir.ActivationFunctionType.Exp)

                _gather_t(0)
                _gather_t(1)
                nc.sync.dma_start(
                    wqka[:].rearrange("p (c n) -> p c n", c=6),
                    bass.AP(qkvw_d.tensor, 0,
                            [[3 * CDIM, 128], [128 * 3 * CDIM, 6], [1, 1536]]))
                nc.sync.dma_start(
                    qkb[:], bass.AP(qkb_d.tensor, 0, [[1, 128], [128, 12]]))
                nc.sync.dma_start(
                    outb[:], bass.AP(outb_d.tensor, 0, [[1, 128], [128, 6]]))
                for t in range(2, 5):
                    _gather_t(t)
                # wot load last on SP before y stores
                nc.sync.dma_start(
                    wota[:].rearrange("p (c n) -> p c n", c=6),
                    bass.AP(outw_d.tensor, 0,
                            [[CDIM, 128], [128 * CDIM, 6], [1, CDIM]]))

                # ---- phase C: v (no bias; v-bias folded into out_b_eff) ----
                for b in range(BLOC):
                    for half in range(2):
                        for t, (k0, pw) in enumerate(KTILES):
                            vtile = vt[b][t]
                            ps = ps_v.tile([128, 384], F32, tag="psv", name="psv")
                            for c in range(6):
                                nc.tensor.matmul(
                                    ps[0:pw, :],
                                    xT(c, b)[:, k0:k0 + pw],
                                    wv(c, half),
                                    start=(c == 0), stop=(c == 5))
                            dst = vtile[0:pw, :].rearrange(
                                "p (h d) -> p h d",
                                h=NH)[:, half * 6:(half + 1) * 6, 0:64]
                            src = ps[0:pw, :].rearrange("p (h d) -> p h d", d=64)
                            nc.gpsimd.tensor_copy(dst, src)
                            if half == 1:
                                nc.gpsimd.memset(
                                    vtile[0:pw, :].rearrange(
                                        "p (h d) -> p h d",
                                        h=NH)[:, :, 64:128], 1.0)

            # ======== phases B + D interleaved per head-pair ========
            with tc.tile_pool(name="ps_qk", bufs=2, space="PSUM") as ps_qk, \
                 tc.tile_pool(name="ps_sT", bufs=4, space="PSUM") as ps_sT, \
                 tc.tile_pool(name="ps_OT", bufs=2, space="PSUM") as ps_OT, \
                 tc.tile_pool(name="es_pool", bufs=8) as es_pool, \
                 tc.tile_pool(name="p_pool", bufs=8) as p_pool:
                for jp in range(6):
                    # ---- B: produce qT[jp], kT[jp] ----
                    for r in (jp, jp + 6):
                        wcol0 = r * 128
                        dst = qT[r] if r < 6 else kT[r - 6]
                        for nb0, nbw in NBLK_B:
                            bb, loc = nb0 // NTOK, nb0 % NTOK
                            ps = ps_qk.tile([128, 386], F32, tag="psqk",
                                            name="psqk")
                            for c in range(6):
                                nc.tensor.matmul(ps[0:128, 0:nbw],
                                                 wqk(c)[:, wcol0:wcol0 + 128],
                                                 xT(c, bb)[:, loc:loc + nbw],
                                                 start=(c == 0), stop=(c == 5))
                            nc.vector.tensor_scalar_add(
                                dst[:, nb0:nb0 + nbw], ps[0:128, 0:nbw],
                                qkb[:, r:r + 1])
                    # ---- D: attention for both batches / both q-blocks ----
                    for b in range(BLOC):
                        for (qstart, qN) in QBLOCKS:
                            qend = qstart + qN
                            valid_t = [t for t in range(5) if QLO[t] < qend]
                            tlast = valid_t[-1]
                            psO = [ps_OT.tile([128, 456], F32, tag="psOT",
                                              name="psOT") for _ in range(2)]
                            # (att_tmp removed: denominator pre-replicated)
                            for t in valid_t:
                                k0, pw = KTILES[t]
                                qlo = max(qstart, QLO[t])
                                off = qlo - qstart
                                Nt = qend - qlo
                                ebase = qlo - QLO[t]
                                psS = [ps_sT.tile([128, 456], F32, tag="psS",
                                                  name="psS") for _ in range(2)]
                                for side in range(2):
                                    r0 = side * 64
                                    nc.tensor.matmul(
                                        psS[side][0:pw, 0:Nt],
                                        kT[jp][r0:r0 + 64,
                                               b * NTOK + k0:b * NTOK + k0 + pw],
                                        qT[jp][r0:r0 + 64,
                                               b * NTOK + qlo:b * NTOK + qlo + Nt],
                                        start=True, stop=True,
                                        tile_position=(r0, 0))
                                for side in range(2):
                                    h = 2 * jp + side
                                    es = es_pool.tile([128, 456], F16, tag="es",
                                                      name="es")
                                    nc.scalar.activation(
                                        es[0:pw, 0:Nt], psS[side][0:pw, 0:Nt],
                                        mybir.ActivationFunctionType.Exp)
                                    p = p_pool.tile([128, 456], F16, tag="p",
                                                    name="p")
                                    nc.vector.tensor_tensor(
                                        out=p[0:pw, 0:Nt],
                                        in0=es[0:pw, 0:Nt],
                                        in1=expb[t][0:pw,
                                                    h * WID[t] + ebase:
                                                    h * WID[t] + ebase + Nt],
                                        op=mybir.AluOpType.mult)
                                    nc.tensor.matmul(
                                        psO[side][0:128, off:off + Nt],
                                        vt[b][t][0:pw, h * 128:(h + 1) * 128],
                                        p[0:pw, 0:Nt],
                                        start=(t == valid_t[0]),
                                        stop=(t == tlast))
                            for side in range(2):
                                r0 = side * 64
                                nc.vector.tensor_tensor(
                                    out=oT[jp][r0:r0 + 64,
                                               b * NTOK + qstart:b * NTOK + qend],
                                    in0=psO[side][0:64, 0:qN],
                                    in1=psO[side][64:128, 0:qN],
                                    op=mybir.AluOpType.divide)

        # ================= phase E: output projection (yT layout) ============
        with tc.tile_pool(name="ps_o", bufs=3, space="PSUM") as ps_o, \
             tc.tile_pool(name="out_sb", bufs=1) as out_sb:
            ySB = [out_sb.tile([128, NSEQ], F16, tag=f"ySB{o}", name=f"ySB{o}")
                   for o in range(6)]
            for o in range(6):
                for nb0, nbw in NBLK:
                    ps = ps_o.tile([128, 386], F32, tag="pso", name="pso")
                    for c in range(6):
                        nc.tensor.matmul(
                            ps[0:128, 0:nbw],
                            wot(c)[:, o * 128:(o + 1) * 128],
                            oT[c][:, nb0:nb0 + nbw],
                            start=(c == 0), stop=(c == 5))
                    nc.scalar.activation(
                        ySB[o][:, nb0:nb0 + nbw], ps[0:128, 0:nbw],
                        mybir.ActivationFunctionType.Identity,
                        bias=outb[:, o:o + 1])
                    nc.sync.dma_start(
                        y_d[o * 128:(o + 1) * 128, nb0:nb0 + nbw],
                        ySB[o][:, nb0:nb0 + nbw])


def kernel(x, qkv_w, qkv_b, pos_emb, out_w, out_b, rel_index):
    x = np.asarray(x, dtype=np.float32)
    qkv_w = np.asarray(qkv_w, dtype=np.float32)
    qkv_b = np.asarray(qkv_b, dtype=np.float32)
    pos_emb = np.asarray(pos_emb, dtype=np.float32)
    out_w = np.asarray(out_w, dtype=np.float32)
    out_b = np.asarray(out_b, dtype=np.float32)
    ri = np.asarray(rel_index, dtype=np.int32)

    key = ri.tobytes()
    if key not in _CACHE:
        _CACHE[key] = _build(ri)
    nc = _CACHE[key]

    t3m, pos_embT = _host_prep(pos_emb, ri)
    # fold the attention scale into the q columns of qkv_w / qkv_b
    qkvw_h = qkv_w.astype(np.float16)
    qkvw_h[:, 0:CDIM] = (qkv_w[:, 0:CDIM] * SCALE).astype(np.float16)
    qkb_prep = qkv_b[0:2 * CDIM].astype(np.float32).copy()
    qkb_prep[0:CDIM] *= SCALE
    # v-bias commutes through the (normalized) attention into out projection
    outb_eff = (out_b + qkv_b[2 * CDIM:3 * CDIM] @ out_w).astype(np.float32)

    in_maps = []
    for c in range(NCORES):
        shard = np.ascontiguousarray(
            x[c * BLOC:(c + 1) * BLOC].reshape(NSEQ, CDIM).T.astype(np.float16))
        in_maps.append({
            "xT_in": shard,
            "qkv_w_h": qkvw_h,
            "qkb_prep": qkb_prep,
            "t3m": t3m,
            "pos_embT": pos_embT,
            "out_w_h": out_w.astype(np.float16),
            "out_b_eff": outb_eff,
        })
    res = run_bass_kernel_spmd(nc, in_maps, core_ids=list(range(NCORES)))
    out = np.empty((B, NTOK, CDIM), dtype=np.float32)
    for c in range(NCORES):
        yT = res.results[c]["yT"].astype(np.float32)     # [CDIM, NSEQ]
        out[c * BLOC:(c + 1) * BLOC] = yT.T.reshape(BLOC, NTOK, CDIM)
    return out


# revision 37
# speedup vs baseline: 1.1350x; 1.0225x over previous
"""Trainium2 Bass kernel for nn_EnhanceSelfAttention (B=16, N=577, C=768, H=12).

Self-contained: takes full unsharded inputs, shards batch across 8 NeuronCores
(2 batches/core), runs a fused attention kernel per core, gathers the output.

Per-core pipeline (f16 matmul operands, fp32 PSUM accumulation):
  Host prep: xT (pre-transposed f16 x shard), qkv_w f16 with the q-columns
  pre-scaled by d^-0.5, q-bias pre-scaled likewise, out_b folded with the
  v-bias contribution (attn rows sum to 1, so v-bias commutes to out_b via
  qkv_b[2C:] @ out_w), expanded bias table t3m (Toeplitz gather, causal mask
  folded as -65504).

  C. v = xT.T @ wv per causal k-tile, [k, 12*65] f16 with a ones column
     per head (softmax-denominator trick).  psum->sbuf casts on DVE.
  G. gather bias: 24 strided DMAs from t3m (1.1KB lines) -> exp+reorder on
     ACT into per-k-tile [k, h-major] f16 exp(bias) tiles (masked -> 0).
  B. qT,kT = wqk.T @ xT per head-pair, psum->sbuf + bias on DVE
     (tensor_scalar_add, scale pre-folded into weights).
  D. per head-pair: sT = kT.T@qT, both heads into one 2-bank psum tile;
     one fused exp (ACT) -> es[k, 2, q]; one fused mult with the 2-head
     expb slice (DVE, 2x mode); OT += v.T@p over causal k-tiles in PSUM.
     Row 64 = denominator; partition_broadcast + divide on GpSimd.
  E. yT = out_w.T @ oT (dim-major output), psum->sbuf + out_b_eff bias on
     ACT, f16 yT streamed to DRAM; host transposes back.
"""

import numpy as np
import ml_dtypes

import concourse.bass as bass
import concourse.tile as tile
from concourse import bacc, mybir
from concourse.bass_utils import run_bass_kernel_spmd

F32 = mybir.dt.float32
BF16 = mybir.dt.bfloat16
F16 = mybir.dt.float16

B, NTOK, CDIM, NH, DH = 16, 577, 768, 12, 64
GRID = 24
NRD = (2 * GRID - 1) * (2 * GRID - 1) + 3  # 2212
NCORES = 8
BLOC = B // NCORES       # batches per core
NSEQ = BLOC * NTOK       # 1154
SCALE = DH ** -0.5       # 0.125
NEG = -65504.0

QBLOCKS = [(0, 121), (121, 456)]            # (qstart, qN)
# k-tiles: (k0, pw).  t=0: partitions 0..120 <-> k=0..120 (incl cls col k=0)
KTILES = [(0, 121), (121, 120), (241, 120), (361, 120), (481, 96)]
# per-tile stored q range: [QLO[t] .. 577)
QLO = [0, 121, 241, 361, 481]
WID = [NTOK - q for q in QLO]               # 577, 456, 336, 216, 96

# expanded-table strides (T3m[kh, d1, qh, h], d1 = qw-kw+23)
T3_KH = 47 * GRID * NH   # 13536
T3_D1 = GRID * NH        # 288

_CACHE = {}


def _check_rel_index(ri):
    """Assert the Toeplitz structure the gather DMAs rely on."""
    assert ri.shape == (NTOK, NTOK)
    assert ri[0, 0] == NRD - 1
    assert (ri[0, 1:] == NRD - 3).all()
    assert (ri[1:, 0] == NRD - 2).all()
    a = np.arange(NTOK - 1)
    qh, qw = a % GRID, a // GRID
    rel0 = qh[:, None] - qh[None, :] + GRID - 1
    rel1 = qw[:, None] - qw[None, :] + GRID - 1
    expect = rel0 + rel1 * (2 * GRID - 1)
    assert np.array_equal(ri[1:, 1:], expect), "rel_index lacks expected structure"


def _host_prep(pos_emb, rel_index):
    _check_rel_index(np.asarray(rel_index))
    pe_t = np.asarray(pos_emb, dtype=np.float32).T      # [NRD, NH]
    # expanded gather table with the causal mask folded in:
    # T3m[kh, d1, qh, h] = pos_emb[h, (qh-kh+23) + 47*d1] + (NEG if q<k)
    # where q-k = (qh-kh) + 24*(d1-23)
    kh = np.arange(GRID)[:, None, None]
    d1 = np.arange(2 * GRID - 1)[None, :, None]
    qh = np.arange(GRID)[None, None, :]
    ridx = (qh - kh + GRID - 1) + (2 * GRID - 1) * d1   # [24, 47, 24]
    t3m = pe_t[ridx]                                    # [24, 47, 24, NH]
    masked = (qh - kh) + GRID * (d1 - (GRID - 1)) < 0
    t3m = t3m + np.where(masked, NEG, 0.0)[..., None].astype(np.float32)
    t3m = np.ascontiguousarray(t3m.reshape(-1)).astype(ml_dtypes.bfloat16)
    pos_embT = np.ascontiguousarray(pe_t).reshape(-1)
    return t3m, pos_embT


def _build(ri):
    """Build + compile the per-core Bass program."""
    nc = bacc.Bacc("TRN2", target_bir_lowering=False, debug=False)

    xT_d = nc.dram_tensor("xT_in", [CDIM, NSEQ], F16, kind="ExternalInput").ap()
    qkvwh_d = nc.dram_tensor("qkv_w_h", [CDIM, 3 * CDIM], F16,
                             kind="ExternalInput").ap()
    qkb_d = nc.dram_tensor("qkb_prep", [2 * CDIM], F32,
                           kind="ExternalInput").ap()
    t3m_d = nc.dram_tensor("t3m", [GRID * 47 * GRID * NH], BF16,
                           kind="ExternalInput").ap()
    pe_d = nc.dram_tensor("pos_embT", [NRD * NH], F32, kind="ExternalInput").ap()
    outwh_d = nc.dram_tensor("out_w_h", [CDIM, CDIM], F16,
                             kind="ExternalInput").ap()
    outb_d = nc.dram_tensor("out_b_eff", [CDIM], F32, kind="ExternalInput").ap()
    y_d = nc.dram_tensor("yT", [CDIM, NSEQ], F16, kind="ExternalOutput").ap()

    with tile.TileContext(nc) as tc:
        _emit(nc, tc, xT_d, qkvwh_d, qkb_d, t3m_d, pe_d, outwh_d, outb_d, y_d)
    nc.compile()
    return nc


def _emit(nc, tc, xT_d, qkvw_d, qkb_d, t3m_d, pe_d, outw_d, outb_d, y_d):
    from contextlib import ExitStack

    NBLK = [(0, 386), (386, 384), (770, 384)]   # n-blocks (E out proj)
    # B blocks aligned to the batch boundary (577) so each reads one xT tile
    NBLK_B = [(0, 289), (289, 288), (577, 289), (866, 288)]

    with ExitStack() as top:
        persist = top.enter_context(tc.tile_pool(name="persist", bufs=1))
        consts = top.enter_context(tc.tile_pool(name="consts", bufs=1))
        wo_pool = top.enter_context(tc.tile_pool(name="wo", bufs=1))

        # ---- constants (tiny DMAs, SP queue, issued after wqk) ----
        # qk bias columns packed [128, 12] (host pre-scaled the q half)
        qkb = consts.tile([128, 12], F32, tag="qkb", name="qkb")
        # out_b_eff columns packed [128, 6]
        outb = consts.tile([128, 6], F32, tag="outb", name="outb")

        # ---- persistent activation storage (all f16) ----
        qT = [persist.tile([128, NSEQ], F16, tag=f"qT{j}", name=f"qT{j}")
              for j in range(6)]
        kT = [persist.tile([128, NSEQ], F16, tag=f"kT{j}", name=f"kT{j}")
              for j in range(6)]
        # per head: 64 value cols + 64 ones cols -> AV rows 64:128 hold the
        # softmax denominator already replicated across partitions (no
        # partition_broadcast needed before the divide)
        vt = [[persist.tile([128, NH * 128], F16, tag=f"v{b}_{t}",
                            name=f"v{b}_{t}")
               for t in range(5)] for b in range(BLOC)]
        oT = [persist.tile([128, NSEQ], F16, tag=f"oT{j}", name=f"oT{j}")
              for j in range(6)]
        expb = [persist.tile([128, WID[t] * NH], F16, tag=f"expb{t}",
                             name=f"expb{t}") for t in range(5)]
        wota = wo_pool.tile([128, 6 * CDIM], F16, tag="wota", name="wota")

        def wot(c):
            return wota[:, c * CDIM:(c + 1) * CDIM]

        # ================= phase 1: projections + gather =================
        with tc.tile_pool(name="xT", bufs=1) as xTp, \
             tc.tile_pool(name="wqk_pool", bufs=1) as wqk_pool, \
             tc.tile_pool(name="stage", bufs=1) as stagep:
            # all 6 row-chunks side by side per tile; xT split per batch and
            # wv per column-half so phase C's first groups depend on the
            # smallest possible DMA set
            xTb = [xTp.tile([128, 6 * NTOK], F16, tag=f"xT{b}", name=f"xT{b}")
                   for b in range(BLOC)]
            wqka = wqk_pool.tile([128, 6 * 1536], F16, tag="wqka", name="wqka")

            def xT(c, b):
                return xTb[b][:, c * NTOK:(c + 1) * NTOK]

            def wqk(c):
                return wqka[:, c * 1536:(c + 1) * 1536]

            # DMA order on SP queue: wv -> xT(b0) -> xT(b1) -> gathers ->
            # wqk -> consts -> wot.  xT split per batch so C(b0) starts early.
            ps_qk = top.enter_context(
                tc.tile_pool(name="ps_qk", bufs=2, space="PSUM"))
            with tc.tile_pool(name="wv_pool", bufs=1) as wv_pool, \
                 tc.tile_pool(name="warm", bufs=1) as warmp, \
                 tc.tile_pool(name="ps_w", bufs=1, space="PSUM") as ps_w, \
                 tc.tile_pool(name="ps_v", bufs=5, space="PSUM") as ps_v:
                wvh = [wv_pool.tile([128, 6 * 384], F16, tag=f"wv{h}",
                                    name=f"wv{h}") for h in range(2)]

                def wv(c, half):
                    return wvh[half][:, c * 384:(c + 1) * 384]

                # startup-critical loads, smallest-dependency-first:
                # wv(half0), xT(b0) -> C(b0,h0) can start; then the rest
                nc.sync.dma_start(
                    wvh[0][:].rearrange("p (c n) -> p c n", c=6),
                    bass.AP(qkvw_d.tensor, 2 * CDIM,
                            [[3 * CDIM, 128], [128 * 3 * CDIM, 6], [1, 384]]))
                nc.sync.dma_start(
                    xTb[0][:].rearrange("p (c n) -> p c n", c=6),
                    bass.AP(xT_d.tensor, 0,
                            [[NSEQ, 128], [128 * NSEQ, 6], [1, NTOK]]))
                nc.sync.dma_start(
                    wvh[1][:].rearrange("p (c n) -> p c n", c=6),
                    bass.AP(qkvw_d.tensor, 2 * CDIM + 384,
                            [[3 * CDIM, 128], [128 * 3 * CDIM, 6], [1, 384]]))
                nc.sync.dma_start(
                    xTb[1][:].rearrange("p (c n) -> p c n", c=6),
                    bass.AP(xT_d.tensor, NTOK,
                            [[NSEQ, 128], [128 * NSEQ, 6], [1, NTOK]]))

                # PE p-state warmup: junk matmuls during the input-DMA window
                # so the 3us clock ramp completes before real work arrives.
                warm = warmp.tile([128, 256], F16, tag="warm", name="warm")
                nc.vector.memset(warm[:], 0.0)
                psw = ps_w.tile([128, 256], F32, tag="psw", name="psw")
                for _ in range(28):
                    nc.tensor.matmul(psw[:], warm[:, 0:128], warm[:],
                                     start=True, stop=True)

                # ---- phase G: bias gather + exp/reorder per k-tile ----
                # SP DMA order: xT, wv, gather(t0), wqk, gather(t1..4), wot
                def _gather_t(t):
                    k0, pw = KTILES[t]
                    qlo = QLO[t]
                    bt = stagep.tile([128, WID[t] * NH], BF16, tag=f"bt{t}",
                                     name=f"bt{t}")
                    klo = max(k0, 1)
                    p0 = klo - k0
                    qg = max(qlo, 1)
                    colg = (qg - qlo) * NH
                    qw0 = (qg - 1) // GRID
                    QR = (NTOK - qg) // GRID
                    KR = (pw - p0) // GRID
                    assert (qg - 1) % GRID == 0 and (klo - 1) % GRID == 0
                    assert (NTOK - qg) % GRID == 0 and (pw - p0) % GRID == 0
                    for kr in range(KR):
                        kw = (klo - 1) // GRID + kr
                        d1_0 = qw0 - kw + (GRID - 1)
                        assert 0 <= d1_0 and d1_0 + QR <= 47
                        src = bass.AP(t3m_d.tensor, d1_0 * T3_D1,
                                      [[T3_KH, GRID], [T3_D1, QR], [1, T3_D1]])
                        dst = bt[p0 + kr * GRID:p0 + (kr + 1) * GRID,
                                 colg:colg + QR * T3_D1].rearrange(
                                     "p (qr i) -> p qr i", i=T3_D1)
                        nc.sync.dma_start(dst, src)
                    if t == 0:
                        # cls column k=0 (partition 0): constant, never masked
                        cnt = NTOK - qg
                        src = bass.AP(pe_d.tensor, (NRD - 2) * NH,
                                      [[0, 1], [0, cnt], [1, NH]])
                        dst = bt[0:1, colg:colg + cnt * NH].rearrange(
                            "p (a h) -> p a h", h=NH)
                        nc.gpsimd.dma_start(dst, src)  # casting DMA f32->bf16
                        # q=0 column: only (0,0) survives; its value cancels
                        # in the softmax normalization
                        nc.gpsimd.memset(bt[0:pw, 0:NH], NEG)
                        nc.gpsimd.memset(bt[0:1, 0:NH], 0.0)
                    # exp + (q,h)->(h,q) reorder on ACT
                    W = WID[t]
                    src = bt[0:pw, 0:W * NH].rearrange("p (q h) -> p q h", h=NH)
                    dst = expb[t][0:pw, :].rearrange("p (h q) -> p q h", h=NH)
                    nc.scalar.activation(dst, src,
                                         mybir.ActivationFunctionType.Exp)

                _gather_t(0)
                _gather_t(1)
                nc.sync.dma_start(
                    wqka[:].rearrange("p (c n) -> p c n", c=6),
                    bass.AP(qkvw_d.tensor, 0,
                            [[3 * CDIM, 128], [128 * 3 * CDIM, 6], [1, 1536]]))
                nc.sync.dma_start(
                    qkb[:], bass.AP(qkb_d.tensor, 0, [[1, 128], [128, 12]]))
                nc.sync.dma_start(
                    outb[:], bass.AP(outb_d.tensor, 0, [[1, 128], [128, 6]]))
                for t in range(2, 5):
                    _gather_t(t)
                # wot load last on SP before y stores
                nc.sync.dma_start(
                    wota[:].rearrange("p (c n) -> p c n", c=6),
                    bass.AP(outw_d.tensor, 0,
                            [[CDIM, 128], [128 * CDIM, 6], [1, CDIM]]))

                # ---- phase C: v (no bias; v-bias folded into out_b_eff) ----
                for b in range(BLOC):
                    for half in range(2):
                        for t, (k0, pw) in enumerate(KTILES):
                            vtile = vt[b][t]
                            ps = ps_v.tile([128, 384], F32, tag="psv", name="psv")
                            for c in range(6):
                                nc.tensor.matmul(
                                    ps[0:pw, :],
                                    xT(c, b)[:, k0:k0 + pw],
                                    wv(c, half),
                                    start=(c == 0), stop=(c == 5))
                            dst = vtile[0:pw, :].rearrange(
                                "p (h d) -> p h d",
                                h=NH)[:, half * 6:(half + 1) * 6, 0:64]
                            src = ps[0:pw, :].rearrange("p (h d) -> p h d", d=64)
                            nc.gpsimd.tensor_copy(dst, src)
                            if half == 1:
                                nc.gpsimd.memset(
                                    vtile[0:pw, :].rearrange(
                                        "p (h d) -> p h d",
                                        h=NH)[:, :, 64:128], 1.0)

            # ======== phases B + D interleaved per head-pair ========
            with tc.tile_pool(name="ps_sT", bufs=4, space="PSUM") as ps_sT, \
                 tc.tile_pool(name="ps_OT", bufs=2, space="PSUM") as ps_OT, \
                 tc.tile_pool(name="es_pool", bufs=8) as es_pool, \
                 tc.tile_pool(name="p_pool", bufs=8) as p_pool:
                for jp in range(6):
                    # ---- B: produce qT[jp], kT[jp] ----
                    for r in (jp, jp + 6):
                        wcol0 = r * 128
                        dst = qT[r] if r < 6 else kT[r - 6]
                        for nb0, nbw in NBLK_B:
                            bb, loc = nb0 // NTOK, nb0 % NTOK
                            ps = ps_qk.tile([128, 386], F32, tag="psqk",
                                            name="psqk")
                            for c in range(6):
                                nc.tensor.matmul(ps[0:128, 0:nbw],
                                                 wqk(c)[:, wcol0:wcol0 + 128],
                                                 xT(c, bb)[:, loc:loc + nbw],
                                                 start=(c == 0), stop=(c == 5))
                            nc.vector.tensor_scalar_add(
                                dst[:, nb0:nb0 + nbw], ps[0:128, 0:nbw],
                                qkb[:, r:r + 1])
                    # ---- D: attention for both batches / both q-blocks ----
                    for b in range(BLOC):
                        for (qstart, qN) in QBLOCKS:
                            qend = qstart + qN
                            valid_t = [t for t in range(5) if QLO[t] < qend]
                            tlast = valid_t[-1]
                            psO = [ps_OT.tile([128, 456], F32, tag="psOT",
                                              name="psOT") for _ in range(2)]
                            # (att_tmp removed: denominator pre-replicated)
                            for t in valid_t:
                                k0, pw = KTILES[t]
                                qlo = max(qstart, QLO[t])
                                off = qlo - qstart
                                Nt = qend - qlo
                                ebase = qlo - QLO[t]
                                psS = [ps_sT.tile([128, 456], F32, tag="psS",
                                                  name="psS") for _ in range(2)]
                                for side in range(2):
                                    r0 = side * 64
                                    nc.tensor.matmul(
                                        psS[side][0:pw, 0:Nt],
                                        kT[jp][r0:r0 + 64,
                                               b * NTOK + k0:b * NTOK + k0 + pw],
                                        qT[jp][r0:r0 + 64,
                                               b * NTOK + qlo:b * NTOK + qlo + Nt],
                                        start=True, stop=True,
                                        tile_position=(r0, 0))
                                for side in range(2):
                                    h = 2 * jp + side
                                    es = es_pool.tile([128, 456], F16, tag="es",
                                                      name="es")
                                    nc.scalar.activation(
                                        es[0:pw, 0:Nt], psS[side][0:pw, 0:Nt],
                                        mybir.ActivationFunctionType.Exp)
                                    p = p_pool.tile([128, 456], F16, tag="p",
                                                    name="p")
                                    nc.vector.tensor_tensor(
                                        out=p[0:pw, 0:Nt],
                                        in0=es[0:pw, 0:Nt],
                                        in1=expb[t][0:pw,
                                                    h * WID[t] + ebase:
                                                    h * WID[t] + ebase + Nt],
                                        op=mybir.AluOpType.mult)
                                    nc.tensor.matmul(
                                        psO[side][0:128, off:off + Nt],
                                        vt[b][t][0:pw, h * 128:(h + 1) * 128],
                                        p[0:pw, 0:Nt],
                                        start=(t == valid_t[0]),
                                        stop=(t == tlast))
                            for side in range(2):
                                r0 = side * 64
                                nc.vector.tensor_tensor(
                                    out=oT[jp][r0:r0 + 64,
                                               b * NTOK + qstart:b * NTOK + qend],
                                    in0=psO[side][0:64, 0:qN],
                                    in1=psO[side][64:128, 0:qN],
                                    op=mybir.AluOpType.divide)

        # ================= phase E: output projection (yT layout) ============
        with tc.tile_pool(name="ps_o", bufs=3, space="PSUM") as ps_o, \
             tc.tile_pool(name="out_sb", bufs=1) as out_sb:
            ySB = [out_sb.tile([128, NSEQ], F16, tag=f"ySB{o}", name=f"ySB{o}")
                   for o in range(6)]
            for o in range(6):
                for nb0, nbw in NBLK:
                    ps = ps_o.tile([128, 386], F32, tag="pso", name="pso")
                    for c in range(6):
                        nc.tensor.matmul(
                            ps[0:128, 0:nbw],
                            wot(c)[:, o * 128:(o + 1) * 128],
                            oT[c][:, nb0:nb0 + nbw],
                            start=(c == 0), stop=(c == 5))
                    nc.scalar.activation(
                        ySB[o][:, nb0:nb0 + nbw], ps[0:128, 0:nbw],
                        mybir.ActivationFunctionType.Identity,
                        bias=outb[:, o:o + 1])
                    nc.sync.dma_start(
                        y_d[o * 128:(o + 1) * 128, nb0:nb0 + nbw],
                        ySB[o][:, nb0:nb0 + nbw])


def kernel(x, qkv_w, qkv_b, pos_emb, out_w, out_b, rel_index):
    x = np.asarray(x, dtype=np.float32)
    qkv_w = np.asarray(qkv_w, dtype=np.float32)
    qkv_b = np.asarray(qkv_b, dtype=np.float32)
    pos_emb = np.asarray(pos_emb, dtype=np.float32)
    out_w = np.asarray(out_w, dtype=np.float32)
    out_b = np.asarray(out_b, dtype=np.float32)
    ri = np.asarray(rel_index, dtype=np.int32)

    key = ri.tobytes()
    if key not in _CACHE:
        _CACHE[key] = _build(ri)
    nc = _CACHE[key]

    t3m, pos_embT = _host_prep(pos_emb, ri)
    # fold the attention scale into the q columns of qkv_w / qkv_b
    qkvw_h = qkv_w.astype(np.float16)
    qkvw_h[:, 0:CDIM] = (qkv_w[:, 0:CDIM] * SCALE).astype(np.float16)
    qkb_prep = qkv_b[0:2 * CDIM].astype(np.float32).copy()
    qkb_prep[0:CDIM] *= SCALE
    # v-bias commutes through the (normalized) attention into out projection
    outb_eff = (out_b + qkv_b[2 * CDIM:3 * CDIM] @ out_w).astype(np.float32)

    in_maps = []
    for c in range(NCORES):
        shard = np.ascontiguousarray(
            x[c * BLOC:(c + 1) * BLOC].reshape(NSEQ, CDIM).T.astype(np.float16))
        in_maps.append({
            "xT_in": shard,
            "qkv_w_h": qkvw_h,
            "qkb_prep": qkb_prep,
            "t3m": t3m,
            "pos_embT": pos_embT,
            "out_w_h": out_w.astype(np.float16),
            "out_b_eff": outb_eff,
        })
    res = run_bass_kernel_spmd(nc, in_maps, core_ids=list(range(NCORES)))
    out = np.empty((B, NTOK, CDIM), dtype=np.float32)
    for c in range(NCORES):
        yT = res.results[c]["yT"].astype(np.float32)     # [CDIM, NSEQ]
        out[c * BLOC:(c + 1) * BLOC] = yT.T.reshape(BLOC, NTOK, CDIM)
    return out
